# revision 15
# baseline (speedup 1.0000x reference)
"""Trainium2 Bass kernel for nn_MemristiveLinear.

The reference's differential-conductance-pair math collapses exactly:
  g_pos - g_neg = k_cond * weights   (the G_OFF leak terms cancel)
so total_currents = K_V * inputs @ (k_cond * weights) and
  y = total_currents / (K_V * k_cond) = inputs @ weights = x @ w + b.

Device kernel: yT = w_shard.T @ x_shardT in bf16 (f32 PSUM accumulate),
sharded over 8 NeuronCores in a 2 (batch) x 4 (n_out) grid; the bias is
added on the host (a [512] vector broadcast, negligible), so the device
does a pure matmul.

The kernel is ~4-6us and latency-dominated (fixed DMA issue/completion
costs, not bandwidth), so the structure is built around that:
 - raw bass with hand-placed semaphores (no TileContext) to avoid the
   Tile framework's start/end all-engine barrier overhead.
 - inputs cast host-side to bf16 (rel err ~2.9e-3 vs the 2e-2 gate),
   halving the per-core input DMA from 768KB to 384KB; output returned
   as bf16 too (64KB).
 - one packed input DRAM tensor per core, [128, 1536] bf16, laid out
   per-partition as [w0|x0|w1|x1|w2|x2|w3|x3] (ko = 128-deep contraction
   blocks), split into 2 dma_starts at elem 920 so matmuls for ko 0,1
   overlap the tail of the input transfer (ko 2,3 gate on chunk B).
 - output via a single HWDGE dma_start after the PSUM->SBUF copy.
   (A prepared SWDGE kv_writeback + trigger_dma was tried to skip the
   HWDGE issue latency; on real HW the per-use Q7 cost makes it ~1.3us
   SLOWER per iteration than the plain HWDGE path, so it was dropped.)
 - dummy matmuls on scratch SBUF warm the PE HAM throttle while the
   input streams in.
"""

import numpy as np
import ml_dtypes

import concourse.bacc as bacc
import concourse.mybir as mybir
from concourse.bass_utils import run_bass_kernel_spmd

BF16 = ml_dtypes.bfloat16

N_CORES = 8
B, NIN, NOUT = 512, 512, 512
GB, GN = 2, 4                  # batch groups x n_out groups
BS, NS = B // GB, NOUT // GN   # 256 batch rows, 128 n_out cols per core
P = 128
KO = NIN // P                  # 4 contraction blocks
CHUNK = NS + BS                # 384 elems per ko chunk (w block + x block)
INW = KO * CHUNK               # 1536 elems per partition
SPLIT_ELEMS = 920              # input DMA split point (elems/partition);
                               # ko 0,1 (elems 0..768) are fully inside
                               # the first chunk.
GATE_KO = 2                    # first ko gated by the second input DMA

_NC = None


def _build(n_iters=1, chain=False, split=SPLIT_ELEMS, warmup=4):
    """chain=True serializes iterations (iter i's input DMAs wait on iter
    i-1's output completion) for serial-latency wall-clock measurement."""
    nc = bacc.Bacc("TRN2", target_bir_lowering=False, debug=False,
                   num_devices=N_CORES)
    bf16 = mybir.dt.bfloat16
    f32 = mybir.dt.float32
    inp = nc.dram_tensor("inp", [P, INW], bf16, kind="ExternalInput")
    y = nc.dram_tensor("y", [NS, BS], bf16, kind="ExternalOutput")
    with (
        nc.semaphore("s_a") as s_a,
        nc.semaphore("s_b") as s_b,
        nc.semaphore("s_scr") as s_scr,
        nc.semaphore("s_pe") as s_pe,
        nc.semaphore("s_dve") as s_dve,
        nc.semaphore("s_out") as s_out,
        nc.sbuf_tensor("t_in", [P, INW], bf16) as t_in,
        nc.sbuf_tensor("t_out", [NS, BS], bf16) as t_out,
        nc.sbuf_tensor("t_scr", [P, 512], bf16) as t_scr,
    ):
        ps = nc.alloc_psum_tensor("ps", [NS, BS], f32)
        ps_d = nc.alloc_psum_tensor("ps_d", [P, 512], f32)

        # zero the warmup scratch so nothing ever reads uninitialized SBUF
        if warmup:
            nc.vector.memset(t_scr[:, :], 0).then_inc(s_scr, 1)
            nc.tensor.wait_ge(s_scr, 1)

        for it in range(n_iters):
            if chain and it > 0:
                # serialize iterations for latency measurement
                nc.sync.wait_ge(s_out, 16 * it)

            # input DMAs (HWDGE via SP); split >= INW means a single DMA
            nc.sync.dma_start(t_in[:, 0:min(split, INW)],
                              inp.ap()[:, 0:min(split, INW)]).then_inc(s_a, 16)
            if split < INW:
                nc.sync.dma_start(t_in[:, split:INW],
                                  inp.ap()[:, split:INW]).then_inc(s_b, 16)

            # PE warmup on scratch while input streams in (first iter only;
            # in chained timing runs the PE stays warm across iterations)
            if it == 0:
                for _ in range(warmup):
                    nc.tensor.matmul(ps_d.ap(), t_scr[:, 0:128],
                                     t_scr[:, 0:512], start=True, stop=True)

            # real matmuls: ps[NS, BS] += w_ko.T @ x_ko
            nc.tensor.wait_ge(s_a, 16 * (it + 1))
            mm = None
            for ko in range(KO):
                base = ko * CHUNK
                if ko == GATE_KO and split < INW:
                    nc.tensor.wait_ge(s_b, 16 * (it + 1))
                mm = nc.tensor.matmul(
                    ps.ap(),
                    t_in[:, base:base + NS],
                    t_in[:, base + NS:base + CHUNK],
                    start=(ko == 0),
                    stop=(ko == KO - 1),
                )
            mm.then_inc(s_pe, 1)

            # PSUM -> SBUF, cast f32 -> bf16
            nc.vector.wait_ge(s_pe, it + 1)
            nc.vector.tensor_scalar_mul(t_out[:, :], ps.ap(),
                                        1.0).then_inc(s_dve, 1)

            # output DMA
            nc.sync.wait_ge(s_dve, it + 1)
            nc.sync.dma_start(y.ap(), t_out[:, :]).then_inc(s_out, 16)

        # drain: the NEFF must not end before the last output landed
        nc.sync.wait_ge(s_out, 16 * n_iters)
    nc.compile()
    return nc


def _get_nc():
    global _NC
    if _NC is None:
        _NC = _build()
    return _NC


def _make_in_maps(x, w, b):
    xb = np.asarray(x, dtype=np.float32).astype(BF16)
    wb = np.asarray(w, dtype=np.float32).astype(BF16)
    xbT = np.ascontiguousarray(xb.T)               # [NIN, B]
    wr = wb.reshape(KO, P, NOUT)                   # [ko, p, nout]
    xr = xbT.reshape(KO, P, B)                     # [ko, p, batch]
    in_maps = []
    for c in range(N_CORES):
        gb, gn = divmod(c, GN)
        pack = np.empty((P, KO, CHUNK), BF16)
        pack[:, :, 0:NS] = wr[:, :, gn * NS:(gn + 1) * NS].transpose(1, 0, 2)
        pack[:, :, NS:CHUNK] = xr[:, :, gb * BS:(gb + 1) * BS].transpose(1, 0, 2)
        in_maps.append({"inp": pack.reshape(P, INW)})
    return in_maps


def _gather(results, b):
    y = np.empty((B, NOUT), np.float32)
    for c in range(N_CORES):
        gb, gn = divmod(c, GN)
        yt = np.asarray(results[c]["y"]).reshape(NS, BS).astype(np.float32)
        y[gb * BS:(gb + 1) * BS, gn * NS:(gn + 1) * NS] = yt.T
    return y + np.asarray(b, dtype=np.float32)[None, :]


def run(x, w, b, **spmd_kwargs):
    """Run on hardware; returns (y, BassKernelResults)."""
    nc = _get_nc()
    res = run_bass_kernel_spmd(nc, _make_in_maps(x, w, b),
                               list(range(N_CORES)), **spmd_kwargs)
    return _gather(res.results, b), res


def kernel(x, w, b):
    y, _ = run(x, w, b)
    return y
